# revision 24
# baseline (speedup 1.0000x reference)
"""Trainium2 Bass kernel for group-quant (fake int8, V=64) + Linear.

reference math (per row of x):
    absmax over feature-groups of 64 -> delta = max(2*absmax/254, 1e-5)
    xq = clip(round(x/delta), -127, 127) * delta      (fake quant)
    out = xq @ W.T + b

Sharding (v6): 2-D — tokens 4-way x out-features 2-way across 8 cores.
Each core handles 2048 tokens x 2048 out-features; its W^T shard
([4096, 2048] fp16, pre-packed on host) is 128 KB/partition and stays
FULLY resident in SBUF: single phase, no W re-load, no x~^T spill.

x is cast to fp16 on the host: |x| <= ~6 so fp16's 10-bit mantissa
keeps the quant decisions almost always identical to fp32 (measured
end-to-end rel err 2.5e-3 vs the 2e-2 budget).  This halves x HBM
traffic to 16.8 MB/core — the first ~100us are DMA-engine bound (the
W shard + x + XBAR transposes saturate the ~358 GB/s per-core fabric),
so x bytes trade 1:1 against how fast W can land.  The quant ARITHMETIC
stays fp32: strided/broadcast DVE ops on fp16 measured 2-10x SLOWER
than fp32 (no 16-bit fast path for those APs), so each half is scaled
fp16->fp32 into a scratch (upconvert fused into the multiply), rounded
flat on the ACT engine (2 Copy+bias passes; fp32 internal math makes
+/-1.5*2^23 an exact RNE), and dequanted fp32->fp16 back into the x
buffer (that mixed broadcast pattern measured fast), then transposed.

Device schedule per core:
  A burst of dummy matmuls, gated on the first x tile's DMA (so the
  HAM clock-gate warm-up isn't wasted waiting), brings the PE to full
  clock right before real matmuls start.  Matmuls run k-outer/oc-inner
  (4 matmuls share one stationary), accumulating into 4 PSUM banks
  ping-ponged across token tiles.  The first four token tiles run
  oc-pair-staggered segments so the W stream (sync queue, nothing else
  on it) stays ahead of demand: chunks 2/3 are not needed until
  ~62/69us.  PSUM is evacuated by ACT copies; output DMAs follow on
  the scalar queue (bias added on host).
"""

import numpy as np

import concourse.bass as bass
import concourse.mybir as mybir
import concourse.tile as tile
from concourse.bass_utils import run_bass_kernel_spmd

N_CORES = 8
TP = 4                     # token-parallel ways
OP = 2                     # out-feature-parallel ways
MAGIC = 1.5 * 2.0**23      # fp32 round-to-nearest-even constant
QSCALE = 1.0 / 127.0       # 2/(qmax-qmin) with qmax=127, qmin=-127
DELTA_MIN = 1e-5


def _split_multiwait(nc):
    """This walrus build allows at most ONE sync wait per instruction
    ("Too many sync wait commands", CoreV3GenImpl setupSyncWait) and none
    on Drain. Tile freely attaches several waits to one instruction, so
    post-process: move excess waits onto single-wait NoOps inserted just
    before the instruction on the same engine queue (semantics identical —
    the queue stalls at the nop instead of at the instruction)."""
    nid = 0
    for fn in nc.m.functions:
        for bb in fn.blocks:
            insts = list(bb.instructions)
            out = []
            changed = False
            for inst in insts:
                si = inst.sync_info
                waits = list(si.on_wait) if si is not None and si.on_wait else []
                limit = 0 if type(inst).__name__ == "InstDrain" else 1
                if len(waits) > limit:
                    changed = True
                    keep = waits[len(waits) - limit :] if limit else []
                    for w in waits[: len(waits) - limit]:
                        nid += 1
                        out.append(
                            mybir.InstNoOp(
                                name=f"WSPLIT-{nid}",
                                engine=inst.engine,
                                bass_nofuse=True,
                                ins=[],
                                outs=[],
                                sync_info=mybir.SyncInfo(on_wait=[w], on_update=[]),
                            )
                        )
                    si.on_wait = keep
                out.append(inst)
            if changed:
                try:
                    bb.instructions = out
                except Exception:
                    bb.instructions[:] = out


def build(T=2048, K=4096, O=2048, V=64, GVH=12, wq_split=4, split=True,
          warm_mms=40, stag=5):
    f32, f16 = mybir.dt.float32, mybir.dt.float16
    P = 128
    G = K // V                 # quant groups per row (64)
    GH = G // 2                # groups per half (32)
    H = K // 2                 # cols per half (2048)
    KT = K // P                # contraction tiles (32)
    NT = T // P                # token tiles per core (16)
    OC = 512                   # oc chunk (psum bank width fp32)
    NOC = O // OC              # 4
    KQW = KT // wq_split       # k-tiles per W DMA quarter (8)

    nc = bass.Bass()
    x = nc.dram_tensor("x", [T, K], f16, kind="ExternalInput")
    wt = nc.dram_tensor("wt", [NOC, P, KT * OC], f16, kind="ExternalInput")
    out = nc.dram_tensor("out", [T, O], f32, kind="ExternalOutput")

    mult = mybir.AluOpType.mult
    amax_op = mybir.AluOpType.max

    with tile.TileContext(nc) as tc:
        with (
            tc.tile_pool(name="x", bufs=2) as pool_x,
            tc.tile_pool(name="q32", bufs=2) as pool_q,
            tc.tile_pool(name="st", bufs=2) as pool_s,
            tc.tile_pool(name="xt", bufs=5) as pool_xt,
            tc.tile_pool(name="w", bufs=1) as pool_w,
            tc.tile_pool(name="o", bufs=2) as pool_o,
            tc.tile_pool(name="ps", bufs=1, space="PSUM") as pool_ps,
        ):
            # ---- W shard loads: sync queue carries ONLY these ----
            def post_w(oc):
                wtile = pool_w.tile([P, KT, OC], f16, tag=f"w{oc}", name=f"w{oc}")
                for q in range(wq_split):
                    nc.sync.dma_start(
                        out=wtile[:, q * KQW : (q + 1) * KQW, :].rearrange(
                            "p k o -> p (k o)"
                        ),
                        in_=wt[oc][:, q * KQW * OC : (q + 1) * KQW * OC],
                    )
                return wtile

            wcur = [post_w(oc) for oc in range(NOC)]

            # ---- quant tile 0 DMA first (quarters, so its chain starts
            # the moment the first 0.25 MB lands) ----
            xq0 = pool_x.tile([P, K], f16, tag="x", name="x0")
            Q4 = K // 4
            for qq in range(4):
                nc.gpsimd.dma_start(
                    out=xq0[:, qq * Q4 : (qq + 1) * Q4],
                    in_=x[0:P, qq * Q4 : (qq + 1) * Q4],
                )

            # ---- PE warm-up: dummy matmuls gated on tile 0's first quant
            # chunk (copy creates the data dep), into a PSUM bank whose
            # first real use is late.  The first transpose can only reach
            # the PE ~33us in (the Activation hwdge queue starts ~32us
            # into every kernel), so the warm-up spans ~21-30us and the
            # HAM clock-gate is at 8/8 right when real matmuls start ----
            dummy = pool_w.tile([P, OC], f16, tag="warm", name="warm")

            def emit_warm(q32_gate):
                nc.gpsimd.memset(dummy[:], 0.0)
                nc.gpsimd.tensor_copy(out=dummy[:, :P], in_=q32_gate[:, :P])
                wps = pool_ps.tile([P, OC], f32, tag="ps1_3", name="warmps")
                for i in range(warm_mms):
                    nc.tensor.matmul(
                        wps[:], dummy[:, :P], dummy[:],
                        start=(i == 0), stop=(i == warm_mms - 1),
                    )

            # ---- quant: per chunk (halves; quarters for tile 0) — stats
            # from fp16, scale into fp32 scratch, flat per-engine round,
            # dequant fp32->fp16 back into the x buffer, XBAR transpose.
            # NO ACT-engine instructions anywhere in the kernel: the ACT
            # table load blocks the Activation queue until ~40us, so the
            # scalar queue must stay pure-DMA (transposes + output) ----
            def rnd(eng, q32, c0, c1):   # exact fp32 RNE via +/-MAGIC, flat
                eng.tensor_scalar(
                    out=q32[:, c0:c1], in0=q32[:, c0:c1],
                    scalar1=MAGIC, scalar2=MAGIC,
                    op0=mybir.AluOpType.add, op1=mybir.AluOpType.subtract,
                )

            def emit_quant(t, xq_=None, nch=2, after_chunk0=None):
                if xq_ is None:
                    xq_ = pool_x.tile([P, K], f16, tag="x", name=f"x{t}")
                    nc.gpsimd.dma_start(out=xq_[:], in_=x[t * P : (t + 1) * P, :])
                xts_t = pool_xt.tile([P, KT, P], f16, tag="xt", name=f"xts{t}")
                amax = pool_s.tile([P, G], f32, tag="amax", name=f"amax{t}")
                delta = pool_s.tile([P, G], f32, tag="delta", name=f"delta{t}")
                recip = pool_s.tile([P, G], f32, tag="recip", name=f"recip{t}")
                xr = xq_.rearrange("p (g v) -> p g v", v=V)
                CH = K // nch            # cols per chunk
                GC = G // nch            # groups per chunk
                KTC = KT // nch          # k-tiles per chunk
                GVC = (GVH * 2) // nch   # vector-side groups per chunk

                for h in range(nch):
                    g0 = h * GC
                    gs = slice(g0, g0 + GC)
                    # stats straight off the fp16 tile
                    nc.vector.tensor_reduce(
                        out=amax[:, gs], in_=xr[:, gs, :], axis=mybir.AxisListType.X,
                        op=amax_op, apply_absolute_value=True,
                    )
                    nc.vector.tensor_scalar(
                        out=delta[:, gs], in0=amax[:, gs],
                        scalar1=QSCALE, scalar2=DELTA_MIN, op0=mult,
                        op1=amax_op,
                    )
                    nc.vector.reciprocal(out=recip[:, gs], in_=delta[:, gs])

                    # column-split scale/dequant on vector+gpsimd; the
                    # fused +/-MAGIC round runs as ONE flat op on vector
                    # covering the whole chunk (the 2-op tensor_scalar is
                    # pathologically slow on gpsimd, ~15 ns/col, and drags
                    # concurrent vector ops with it)
                    q32 = pool_q.tile([P, H], f32, tag="q", name=f"q{t}_{h}")
                    qr = q32.rearrange("p (g v) -> p g v", v=V)

                    def rmul(eng, l0, l1):   # q32 = x16 * (1/delta)
                        eng.tensor_tensor(
                            out=qr[:, l0:l1, :], in0=xr[:, g0 + l0 : g0 + l1, :],
                            in1=recip[:, g0 + l0 : g0 + l1, None].to_broadcast(
                                (P, l1 - l0, V)), op=mult,
                        )

                    def dmul(eng, l0, l1):   # x16 = round(q32) * delta
                        eng.tensor_tensor(
                            out=xr[:, g0 + l0 : g0 + l1, :], in0=qr[:, l0:l1, :],
                            in1=delta[:, g0 + l0 : g0 + l1, None].to_broadcast(
                                (P, l1 - l0, V)), op=mult,
                        )

                    rmul(nc.vector, 0, GVC)
                    rmul(nc.gpsimd, GVC, GC)
                    rnd(nc.vector, q32, 0, GC * V)
                    dmul(nc.vector, 0, GVC)
                    dmul(nc.gpsimd, GVC, GC)
                    nc.scalar.dma_start_transpose(
                        xts_t[:, h * KTC : (h + 1) * KTC, :],
                        xq_[:, h * CH : (h + 1) * CH],
                    )
                    if h == 0 and after_chunk0 is not None:
                        after_chunk0(q32)
                return xts_t

            # ---- PSUM evac: DVE copy (gpsimd has no PSUM port), then
            # output DMA on the scalar queue (sync queue stays pure-W) ----
            def evac(t, oc, ps):
                ot = pool_o.tile([P, OC], f32, tag="o", name=f"ot{t}_{oc}")
                nc.vector.tensor_copy(out=ot[:], in_=ps[:])
                nc.scalar.dma_start(
                    out=out[t * P : (t + 1) * P, oc * OC : (oc + 1) * OC], in_=ot[:]
                )

            def emit_mm(t, xts_t, ocs, mode):
                if mode == "oc":
                    for oc in ocs:
                        ps = pool_ps.tile([P, OC], f32, tag=f"ps{t % 2}_{oc}",
                                          name=f"ps{t}_{oc}")
                        for kt in range(KT):
                            nc.tensor.matmul(
                                ps[:], xts_t[:, kt, :], wcur[oc][:, kt, :],
                                start=(kt == 0), stop=(kt == KT - 1),
                            )
                        evac(t, oc, ps)
                else:
                    pss = {
                        oc: pool_ps.tile([P, OC], f32, tag=f"ps{t % 2}_{oc}",
                                         name=f"ps{t}_{oc}")
                        for oc in ocs
                    }
                    for kt in range(KT):
                        for oc in ocs:
                            nc.tensor.matmul(
                                pss[oc][:], xts_t[:, kt, :], wcur[oc][:, kt, :],
                                start=(kt == 0), stop=(kt == KT - 1),
                            )
                    for oc in ocs:
                        evac(t, oc, pss[oc])

            # The first `stag` token tiles run oc-pair-staggered so demand
            # for W chunks 2/3 starts only ~62/69us in; "oc" mode on the
            # first segment of each pair lets matmuls start on partially-
            # arrived chunks.  Last tile oc-major to shrink the evac tail.
            ALL = list(range(NOC))
            segs = []
            for t in range(stag):
                segs.append((t, [0, 1], "oc" if t == 0 else "k"))
            for oc in (2, 3):
                for t in range(stag):
                    segs.append((t, [oc], "k"))
            n_phase1 = len(segs)
            for t in range(stag, NT - 1):
                segs.append((t, ALL, "k"))
            segs.append((NT - 1, ALL, "oc"))

            tiles = {0: emit_quant(0, xq0, nch=4, after_chunk0=emit_warm)}
            qnext = 1

            def emit_q_upto(n):
                nonlocal qnext
                while qnext < min(n, NT):
                    tiles[qnext] = emit_quant(qnext)
                    qnext += 1

            emit_q_upto(2)
            for si, (t, ocs, mode) in enumerate(segs):
                if t >= qnext:
                    emit_q_upto(t + 1)
                emit_mm(t, tiles[t], ocs, mode)
                if si < n_phase1:
                    # cap at stag tiles: a later tile's transpose would wait
                    # on xts buffers released only by the [3]-phase segments
                    # and head-block the scalar ring (deadlock)
                    emit_q_upto(min(3 + si, stag))
                else:
                    emit_q_upto(stag + 2 * (si - n_phase1 + 1))

    if split:
        _split_multiwait(nc)
    return nc


_CACHED = {}

# test-harness knobs (kernel() defaults are what the grader uses)
TRACE = False
LAST_RESULT = None
BUILD_KW = {}


def _get_nc(shape_key):
    if shape_key not in _CACHED:
        T, K, O = shape_key
        _CACHED[shape_key] = build(T=T, K=K, O=O, **BUILD_KW)
    return _CACHED[shape_key]


def pack_w(W: np.ndarray, OC: int = 512, P: int = 128) -> np.ndarray:
    # [out,in] -> W^T [in,out] fp16, packed [NOC, P, KT*OC] so each per-core
    # o-chunk W load is one fully contiguous DMA
    K, O = W.shape[1], W.shape[0]
    KT, NOC = K // P, O // OC
    wt = np.ascontiguousarray(W.T).astype(np.float16)         # [K, O]
    z = wt.reshape(KT, P, NOC, OC).transpose(2, 1, 0, 3)      # [NOC, P, KT, OC]
    return np.ascontiguousarray(z.reshape(NOC, P, KT * OC))


def kernel(x: np.ndarray, W: np.ndarray, b: np.ndarray) -> np.ndarray:
    global LAST_RESULT
    n, k = x.shape               # 8192, 4096
    o = W.shape[0]               # 4096
    assert n % TP == 0 and o % OP == 0
    tpc = n // TP                # 2048 tokens per core
    osh = o // OP                # 2048 out-features per core
    nc = _get_nc((tpc, k, osh))

    wtp = pack_w(W)              # [8, 128, 16384]
    ncs = osh // 512             # oc chunks per shard (4)
    xs = np.ascontiguousarray(x.astype(np.float16)).reshape(TP, tpc, k)
    in_maps = []
    for i in range(N_CORES):
        tb, ob = divmod(i, OP)
        in_maps.append(
            {"x": xs[tb], "wt": np.ascontiguousarray(wtp[ob * ncs : (ob + 1) * ncs])}
        )
    res = run_bass_kernel_spmd(nc, in_maps, list(range(N_CORES)), trace=TRACE)
    LAST_RESULT = res
    full = np.empty((n, o), np.float32)
    for i in range(N_CORES):
        tb, ob = divmod(i, OP)
        full[tb * tpc : (tb + 1) * tpc, ob * osh : (ob + 1) * osh] = (
            res.results[i]["out"]
        )
    full += b.astype(np.float32)[None, :]
    return full


# revision 25
# speedup vs baseline: 1.1657x; 1.1657x over previous
"""Trainium2 Bass kernel for group-quant (fake int8, V=64) + Linear.

reference math (per row of x):
    absmax over feature-groups of 64 -> delta = max(2*absmax/254, 1e-5)
    xq = clip(round(x/delta), -127, 127) * delta      (fake quant)
    out = xq @ W.T + b

Sharding (v6): 2-D — tokens 4-way x out-features 2-way across 8 cores.
Each core handles 2048 tokens x 2048 out-features; its W^T shard
([4096, 2048] fp16, pre-packed on host) is 128 KB/partition and stays
FULLY resident in SBUF: single phase, no W re-load, no x~^T spill.

x is cast to fp16 on the host: |x| <= ~6 so fp16's 10-bit mantissa
keeps the quant decisions almost always identical to fp32 (measured
end-to-end rel err 2.5e-3 vs the 2e-2 budget).  This halves x HBM
traffic to 16.8 MB/core — the first ~100us are DMA-engine bound (the
W shard + x + XBAR transposes saturate the ~358 GB/s per-core fabric),
so x bytes trade 1:1 against how fast W can land.  The quant ARITHMETIC
stays fp32: strided/broadcast DVE ops on fp16 measured 2-10x SLOWER
than fp32 (no 16-bit fast path for those APs), so each half is scaled
fp16->fp32 into a scratch (upconvert fused into the multiply), rounded
flat on the ACT engine (2 Copy+bias passes; fp32 internal math makes
+/-1.5*2^23 an exact RNE), and dequanted fp32->fp16 back into the x
buffer (that mixed broadcast pattern measured fast), then transposed.

Device schedule per core:
  A burst of dummy matmuls, gated on the first x tile's DMA (so the
  HAM clock-gate warm-up isn't wasted waiting), brings the PE to full
  clock right before real matmuls start.  Matmuls run k-outer/oc-inner
  (4 matmuls share one stationary), accumulating into 4 PSUM banks
  ping-ponged across token tiles.  The first four token tiles run
  oc-pair-staggered segments so the W stream (sync queue, nothing else
  on it) stays ahead of demand: chunks 2/3 are not needed until
  ~62/69us.  PSUM is evacuated by ACT copies; output DMAs follow on
  the scalar queue (bias added on host).
"""

import numpy as np

import concourse.bass as bass
import concourse.mybir as mybir
import concourse.tile as tile
from concourse.bass_utils import run_bass_kernel_spmd

N_CORES = 8
TP = 4                     # token-parallel ways
OP = 2                     # out-feature-parallel ways
MAGIC = 1.5 * 2.0**23      # fp32 round-to-nearest-even constant
QSCALE = 1.0 / 127.0       # 2/(qmax-qmin) with qmax=127, qmin=-127
DELTA_MIN = 1e-5


def _split_multiwait(nc):
    """This walrus build allows at most ONE sync wait per instruction
    ("Too many sync wait commands", CoreV3GenImpl setupSyncWait) and none
    on Drain. Tile freely attaches several waits to one instruction, so
    post-process: move excess waits onto single-wait NoOps inserted just
    before the instruction on the same engine queue (semantics identical —
    the queue stalls at the nop instead of at the instruction)."""
    nid = 0
    for fn in nc.m.functions:
        for bb in fn.blocks:
            insts = list(bb.instructions)
            out = []
            changed = False
            for inst in insts:
                si = inst.sync_info
                waits = list(si.on_wait) if si is not None and si.on_wait else []
                limit = 0 if type(inst).__name__ == "InstDrain" else 1
                if len(waits) > limit:
                    changed = True
                    keep = waits[len(waits) - limit :] if limit else []
                    for w in waits[: len(waits) - limit]:
                        nid += 1
                        out.append(
                            mybir.InstNoOp(
                                name=f"WSPLIT-{nid}",
                                engine=inst.engine,
                                bass_nofuse=True,
                                ins=[],
                                outs=[],
                                sync_info=mybir.SyncInfo(on_wait=[w], on_update=[]),
                            )
                        )
                    si.on_wait = keep
                out.append(inst)
            if changed:
                try:
                    bb.instructions = out
                except Exception:
                    bb.instructions[:] = out


def build(T=2048, K=4096, O=2048, V=64, GVH=12, wq_split=4, split=True,
          warm_mms=40, stag=5):
    f32, f16 = mybir.dt.float32, mybir.dt.float16
    P = 128
    G = K // V                 # quant groups per row (64)
    GH = G // 2                # groups per half (32)
    H = K // 2                 # cols per half (2048)
    KT = K // P                # contraction tiles (32)
    NT = T // P                # token tiles per core (16)
    OC = 512                   # oc chunk (psum bank width fp32)
    NOC = O // OC              # 4
    KQW = KT // wq_split       # k-tiles per W DMA quarter (8)

    nc = bass.Bass()
    x = nc.dram_tensor("x", [T, K], f16, kind="ExternalInput")
    wt = nc.dram_tensor("wt", [NOC, P, KT * OC], f16, kind="ExternalInput")
    out = nc.dram_tensor("out", [T, O], f32, kind="ExternalOutput")

    mult = mybir.AluOpType.mult
    amax_op = mybir.AluOpType.max

    with tile.TileContext(nc) as tc:
        with (
            tc.tile_pool(name="x", bufs=2) as pool_x,
            tc.tile_pool(name="q32", bufs=2) as pool_q,
            tc.tile_pool(name="st", bufs=2) as pool_s,
            tc.tile_pool(name="xt", bufs=5) as pool_xt,
            tc.tile_pool(name="w", bufs=1) as pool_w,
            tc.tile_pool(name="o", bufs=2) as pool_o,
            tc.tile_pool(name="ps", bufs=1, space="PSUM") as pool_ps,
        ):
            # ---- W shard loads: sync queue carries ONLY these ----
            def post_w(oc):
                wtile = pool_w.tile([P, KT, OC], f16, tag=f"w{oc}", name=f"w{oc}")
                for q in range(wq_split):
                    nc.sync.dma_start(
                        out=wtile[:, q * KQW : (q + 1) * KQW, :].rearrange(
                            "p k o -> p (k o)"
                        ),
                        in_=wt[oc][:, q * KQW * OC : (q + 1) * KQW * OC],
                    )
                return wtile

            wcur = [post_w(oc) for oc in range(NOC)]

            # ---- quant tile 0 DMA first (quarters, so its chain starts
            # the moment the first 0.25 MB lands) ----
            xq0 = pool_x.tile([P, K], f16, tag="x", name="x0")
            Q4 = K // 4
            for qq in range(4):
                nc.gpsimd.dma_start(
                    out=xq0[:, qq * Q4 : (qq + 1) * Q4],
                    in_=x[0:P, qq * Q4 : (qq + 1) * Q4],
                )

            # ---- PE warm-up: dummy matmuls gated on tile 0's first quant
            # chunk (copy creates the data dep), into a PSUM bank whose
            # first real use is late.  The first transpose can only reach
            # the PE ~33us in (the Activation hwdge queue starts ~32us
            # into every kernel), so the warm-up spans ~21-30us and the
            # HAM clock-gate is at 8/8 right when real matmuls start ----
            dummy = pool_w.tile([P, OC], f16, tag="warm", name="warm")

            def emit_warm(q32_gate):
                nc.gpsimd.memset(dummy[:], 0.0)
                nc.gpsimd.tensor_copy(out=dummy[:, :P], in_=q32_gate[:, :P])
                wps = pool_ps.tile([P, OC], f32, tag="ps1_3", name="warmps")
                for i in range(warm_mms):
                    nc.tensor.matmul(
                        wps[:], dummy[:, :P], dummy[:],
                        start=(i == 0), stop=(i == warm_mms - 1),
                    )

            # ---- quant: per chunk (halves; quarters for tile 0) — stats
            # from fp16, scale into fp32 scratch, flat per-engine round,
            # dequant fp32->fp16 back into the x buffer, XBAR transpose.
            # NO ACT-engine instructions anywhere in the kernel: the ACT
            # table load blocks the Activation queue until ~40us, so the
            # scalar queue must stay pure-DMA (transposes + output) ----
            def rnd(eng, q32, c0, c1):   # exact fp32 RNE via +/-MAGIC, flat
                eng.tensor_scalar(
                    out=q32[:, c0:c1], in0=q32[:, c0:c1],
                    scalar1=MAGIC, scalar2=MAGIC,
                    op0=mybir.AluOpType.add, op1=mybir.AluOpType.subtract,
                )

            def emit_quant(t, xq_=None, nch=2, after_chunk0=None,
                           do_transpose=True):
                if xq_ is None:
                    xq_ = pool_x.tile([P, K], f16, tag="x", name=f"x{t}")
                    nc.gpsimd.dma_start(out=xq_[:], in_=x[t * P : (t + 1) * P, :])
                xts_t = pool_xt.tile([P, KT, P], f16, tag="xt", name=f"xts{t}")
                amax = pool_s.tile([P, G], f32, tag="amax", name=f"amax{t}")
                delta = pool_s.tile([P, G], f32, tag="delta", name=f"delta{t}")
                recip = pool_s.tile([P, G], f32, tag="recip", name=f"recip{t}")
                xr = xq_.rearrange("p (g v) -> p g v", v=V)
                CH = K // nch            # cols per chunk
                GC = G // nch            # groups per chunk
                KTC = KT // nch          # k-tiles per chunk
                GVC = (GVH * 2) // nch   # vector-side groups per chunk

                for h in range(nch):
                    g0 = h * GC
                    gs = slice(g0, g0 + GC)
                    # stats straight off the fp16 tile
                    nc.vector.tensor_reduce(
                        out=amax[:, gs], in_=xr[:, gs, :], axis=mybir.AxisListType.X,
                        op=amax_op, apply_absolute_value=True,
                    )
                    nc.vector.tensor_scalar(
                        out=delta[:, gs], in0=amax[:, gs],
                        scalar1=QSCALE, scalar2=DELTA_MIN, op0=mult,
                        op1=amax_op,
                    )
                    nc.vector.reciprocal(out=recip[:, gs], in_=delta[:, gs])

                    # column-split scale/dequant on vector+gpsimd; the
                    # fused +/-MAGIC round runs as ONE flat op on vector
                    # covering the whole chunk (the 2-op tensor_scalar is
                    # pathologically slow on gpsimd, ~15 ns/col, and drags
                    # concurrent vector ops with it)
                    q32 = pool_q.tile([P, H], f32, tag="q", name=f"q{t}_{h}")
                    qr = q32.rearrange("p (g v) -> p g v", v=V)

                    def rmul(eng, l0, l1):   # q32 = x16 * (1/delta)
                        eng.tensor_tensor(
                            out=qr[:, l0:l1, :], in0=xr[:, g0 + l0 : g0 + l1, :],
                            in1=recip[:, g0 + l0 : g0 + l1, None].to_broadcast(
                                (P, l1 - l0, V)), op=mult,
                        )

                    def dmul(eng, l0, l1):   # x16 = round(q32) * delta
                        eng.tensor_tensor(
                            out=xr[:, g0 + l0 : g0 + l1, :], in0=qr[:, l0:l1, :],
                            in1=delta[:, g0 + l0 : g0 + l1, None].to_broadcast(
                                (P, l1 - l0, V)), op=mult,
                        )

                    rmul(nc.vector, 0, GVC)
                    rmul(nc.gpsimd, GVC, GC)
                    rnd(nc.vector, q32, 0, GC * V)
                    dmul(nc.vector, 0, GVC)
                    dmul(nc.gpsimd, GVC, GC)
                    if do_transpose:
                        nc.scalar.dma_start_transpose(
                            xts_t[:, h * KTC : (h + 1) * KTC, :],
                            xq_[:, h * CH : (h + 1) * CH],
                        )
                    if h == 0 and after_chunk0 is not None:
                        after_chunk0(q32)
                return xts_t, xq_

            # ---- PSUM evac: DVE copy (gpsimd has no PSUM port), then
            # output DMA on the scalar queue (sync queue stays pure-W) ----
            def evac(t, oc, ps):
                ot = pool_o.tile([P, OC], f32, tag="o", name=f"ot{t}_{oc}")
                nc.vector.tensor_copy(out=ot[:], in_=ps[:])
                nc.scalar.dma_start(
                    out=out[t * P : (t + 1) * P, oc * OC : (oc + 1) * OC], in_=ot[:]
                )

            def emit_mm(t, xts_t, ocs, mode):
                if mode == "oc":
                    for oc in ocs:
                        ps = pool_ps.tile([P, OC], f32, tag=f"ps{t % 2}_{oc}",
                                          name=f"ps{t}_{oc}")
                        for kt in range(KT):
                            nc.tensor.matmul(
                                ps[:], xts_t[:, kt, :], wcur[oc][:, kt, :],
                                start=(kt == 0), stop=(kt == KT - 1),
                            )
                        evac(t, oc, ps)
                else:
                    pss = {
                        oc: pool_ps.tile([P, OC], f32, tag=f"ps{t % 2}_{oc}",
                                         name=f"ps{t}_{oc}")
                        for oc in ocs
                    }
                    for kt in range(KT):
                        for oc in ocs:
                            nc.tensor.matmul(
                                pss[oc][:], xts_t[:, kt, :], wcur[oc][:, kt, :],
                                start=(kt == 0), stop=(kt == KT - 1),
                            )
                    for oc in ocs:
                        evac(t, oc, pss[oc])

            # The first `stag` token tiles run oc-pair-staggered so demand
            # for W chunks 2/3 starts only ~62/69us in; "oc" mode on the
            # first segment of each pair lets matmuls start on partially-
            # arrived chunks.  Last tile oc-major to shrink the evac tail.
            ALL = list(range(NOC))
            segs = []
            for t in range(stag):
                segs.append((t, [0, 1], "oc" if t == 0 else "k"))
            for oc in (2, 3):
                for t in range(stag):
                    segs.append((t, [oc], "k"))
            n_phase1 = len(segs)
            for t in range(stag, NT - 1):
                segs.append((t, ALL, "k"))
            segs.append((NT - 1, ALL, "oc"))

            tiles = {0: emit_quant(0, xq0, nch=4, after_chunk0=emit_warm)}
            qnext = 1

            def emit_q_upto(n):
                nonlocal qnext
                while qnext < min(n, NT):
                    # tiles >= stag quant early (x-pool paced) but their
                    # transposes are DEFERRED: emitted inline they would
                    # wait on xts buffers released only by the [3]-phase
                    # segments and head-block the scalar ring
                    tiles[qnext] = emit_quant(qnext, do_transpose=qnext < stag)
                    qnext += 1

            tr_next = stag

            def ensure_tr(n):
                nonlocal tr_next
                while tr_next < min(n, NT):
                    xts_t, xq_ = tiles[tr_next]
                    for h in range(2):
                        nc.scalar.dma_start_transpose(
                            xts_t[:, h * (KT // 2) : (h + 1) * (KT // 2), :],
                            xq_[:, h * H : (h + 1) * H],
                        )
                    tr_next += 1

            emit_q_upto(2)
            for si, (t, ocs, mode) in enumerate(segs):
                if t >= qnext:
                    emit_q_upto(t + 1)
                k = si - (n_phase1 - 2)
                if k >= 0:
                    ensure_tr(stag + k + 1)
                emit_mm(t, tiles[t][0], ocs, mode)
                emit_q_upto(3 + si)

    if split:
        _split_multiwait(nc)
    return nc


_CACHED = {}

# test-harness knobs (kernel() defaults are what the grader uses)
TRACE = False
LAST_RESULT = None
BUILD_KW = {}


def _get_nc(shape_key):
    if shape_key not in _CACHED:
        T, K, O = shape_key
        _CACHED[shape_key] = build(T=T, K=K, O=O, **BUILD_KW)
    return _CACHED[shape_key]


def pack_w(W: np.ndarray, OC: int = 512, P: int = 128) -> np.ndarray:
    # [out,in] -> W^T [in,out] fp16, packed [NOC, P, KT*OC] so each per-core
    # o-chunk W load is one fully contiguous DMA
    K, O = W.shape[1], W.shape[0]
    KT, NOC = K // P, O // OC
    wt = np.ascontiguousarray(W.T).astype(np.float16)         # [K, O]
    z = wt.reshape(KT, P, NOC, OC).transpose(2, 1, 0, 3)      # [NOC, P, KT, OC]
    return np.ascontiguousarray(z.reshape(NOC, P, KT * OC))


def kernel(x: np.ndarray, W: np.ndarray, b: np.ndarray) -> np.ndarray:
    global LAST_RESULT
    n, k = x.shape               # 8192, 4096
    o = W.shape[0]               # 4096
    assert n % TP == 0 and o % OP == 0
    tpc = n // TP                # 2048 tokens per core
    osh = o // OP                # 2048 out-features per core
    nc = _get_nc((tpc, k, osh))

    wtp = pack_w(W)              # [8, 128, 16384]
    ncs = osh // 512             # oc chunks per shard (4)
    xs = np.ascontiguousarray(x.astype(np.float16)).reshape(TP, tpc, k)
    in_maps = []
    for i in range(N_CORES):
        tb, ob = divmod(i, OP)
        in_maps.append(
            {"x": xs[tb], "wt": np.ascontiguousarray(wtp[ob * ncs : (ob + 1) * ncs])}
        )
    res = run_bass_kernel_spmd(nc, in_maps, list(range(N_CORES)), trace=TRACE)
    LAST_RESULT = res
    full = np.empty((n, o), np.float32)
    for i in range(N_CORES):
        tb, ob = divmod(i, OP)
        full[tb * tpc : (tb + 1) * tpc, ob * osh : (ob + 1) * osh] = (
            res.results[i]["out"]
        )
    full += b.astype(np.float32)[None, :]
    return full


# revision 26
# speedup vs baseline: 1.2495x; 1.0719x over previous
"""Trainium2 Bass kernel for group-quant (fake int8, V=64) + Linear.

reference math (per row of x):
    absmax over feature-groups of 64 -> delta = max(2*absmax/254, 1e-5)
    xq = clip(round(x/delta), -127, 127) * delta      (fake quant)
    out = xq @ W.T + b

Sharding (v6): 2-D — tokens 4-way x out-features 2-way across 8 cores.
Each core handles 2048 tokens x 2048 out-features; its W^T shard
([4096, 2048] fp16, pre-packed on host) is 128 KB/partition and stays
FULLY resident in SBUF: single phase, no W re-load, no x~^T spill.

x is cast to fp16 on the host: |x| <= ~6 so fp16's 10-bit mantissa
keeps the quant decisions almost always identical to fp32 (measured
end-to-end rel err 2.5e-3 vs the 2e-2 budget).  This halves x HBM
traffic to 16.8 MB/core — the first ~100us are DMA-engine bound (the
W shard + x + XBAR transposes saturate the ~358 GB/s per-core fabric),
so x bytes trade 1:1 against how fast W can land.  The quant ARITHMETIC
stays fp32: strided/broadcast DVE ops on fp16 measured 2-10x SLOWER
than fp32 (no 16-bit fast path for those APs), so each half is scaled
fp16->fp32 into a scratch (upconvert fused into the multiply), rounded
flat on the ACT engine (2 Copy+bias passes; fp32 internal math makes
+/-1.5*2^23 an exact RNE), and dequanted fp32->fp16 back into the x
buffer (that mixed broadcast pattern measured fast), then transposed.

Device schedule per core:
  A burst of dummy matmuls, gated on the first x tile's DMA (so the
  HAM clock-gate warm-up isn't wasted waiting), brings the PE to full
  clock right before real matmuls start.  Matmuls run k-outer/oc-inner
  (4 matmuls share one stationary), accumulating into 4 PSUM banks
  ping-ponged across token tiles.  The first four token tiles run
  oc-pair-staggered segments so the W stream (sync queue, nothing else
  on it) stays ahead of demand: chunks 2/3 are not needed until
  ~62/69us.  PSUM is evacuated by ACT copies; output DMAs follow on
  the scalar queue (bias added on host).
"""

import numpy as np

import concourse.bass as bass
import concourse.mybir as mybir
import concourse.tile as tile
from concourse.bass_utils import run_bass_kernel_spmd

N_CORES = 8
TP = 4                     # token-parallel ways
OP = 2                     # out-feature-parallel ways
MAGIC = 1.5 * 2.0**23      # fp32 round-to-nearest-even constant
QSCALE = 1.0 / 127.0       # 2/(qmax-qmin) with qmax=127, qmin=-127
DELTA_MIN = 1e-5


def _split_multiwait(nc):
    """This walrus build allows at most ONE sync wait per instruction
    ("Too many sync wait commands", CoreV3GenImpl setupSyncWait) and none
    on Drain. Tile freely attaches several waits to one instruction, so
    post-process: move excess waits onto single-wait NoOps inserted just
    before the instruction on the same engine queue (semantics identical —
    the queue stalls at the nop instead of at the instruction)."""
    nid = 0
    for fn in nc.m.functions:
        for bb in fn.blocks:
            insts = list(bb.instructions)
            out = []
            changed = False
            for inst in insts:
                si = inst.sync_info
                waits = list(si.on_wait) if si is not None and si.on_wait else []
                limit = 0 if type(inst).__name__ == "InstDrain" else 1
                if len(waits) > limit:
                    changed = True
                    keep = waits[len(waits) - limit :] if limit else []
                    for w in waits[: len(waits) - limit]:
                        nid += 1
                        out.append(
                            mybir.InstNoOp(
                                name=f"WSPLIT-{nid}",
                                engine=inst.engine,
                                bass_nofuse=True,
                                ins=[],
                                outs=[],
                                sync_info=mybir.SyncInfo(on_wait=[w], on_update=[]),
                            )
                        )
                    si.on_wait = keep
                out.append(inst)
            if changed:
                try:
                    bb.instructions = out
                except Exception:
                    bb.instructions[:] = out


def build(T=2048, K=4096, O=2048, V=64, GVH=12, wq_split=4, split=True,
          warm_mms=40, stag=5):
    f32, f16 = mybir.dt.float32, mybir.dt.float16
    P = 128
    G = K // V                 # quant groups per row (64)
    GH = G // 2                # groups per half (32)
    H = K // 2                 # cols per half (2048)
    KT = K // P                # contraction tiles (32)
    NT = T // P                # token tiles per core (16)
    OC = 512                   # oc chunk (psum bank width fp32)
    NOC = O // OC              # 4
    KQW = KT // wq_split       # k-tiles per W DMA quarter (8)

    nc = bass.Bass()
    x = nc.dram_tensor("x", [T, K], f16, kind="ExternalInput")
    wt = nc.dram_tensor("wt", [NOC, P, KT * OC], f16, kind="ExternalInput")
    out = nc.dram_tensor("out", [T, O], f32, kind="ExternalOutput")

    mult = mybir.AluOpType.mult
    amax_op = mybir.AluOpType.max

    with tile.TileContext(nc) as tc:
        with (
            tc.tile_pool(name="x", bufs=2) as pool_x,
            tc.tile_pool(name="q32", bufs=2) as pool_q,
            tc.tile_pool(name="st", bufs=2) as pool_s,
            tc.tile_pool(name="xt", bufs=5) as pool_xt,
            tc.tile_pool(name="w", bufs=1) as pool_w,
            tc.tile_pool(name="o", bufs=2) as pool_o,
            tc.tile_pool(name="ps", bufs=1, space="PSUM") as pool_ps,
        ):
            # ---- W shard loads: sync queue carries ONLY these ----
            def post_w(oc):
                wtile = pool_w.tile([P, KT, OC], f16, tag=f"w{oc}", name=f"w{oc}")
                for q in range(wq_split):
                    nc.sync.dma_start(
                        out=wtile[:, q * KQW : (q + 1) * KQW, :].rearrange(
                            "p k o -> p (k o)"
                        ),
                        in_=wt[oc][:, q * KQW * OC : (q + 1) * KQW * OC],
                    )
                return wtile

            wcur = [post_w(oc) for oc in range(NOC)]

            # ---- quant tile 0 DMA first (quarters, so its chain starts
            # the moment the first 0.25 MB lands) ----
            xq0 = pool_x.tile([P, K], f16, tag="x", name="x0")
            Q4 = K // 4
            for qq in range(4):
                nc.gpsimd.dma_start(
                    out=xq0[:, qq * Q4 : (qq + 1) * Q4],
                    in_=x[0:P, qq * Q4 : (qq + 1) * Q4],
                )

            # ---- PE warm-up: dummy matmuls gated on tile 0's first quant
            # chunk (copy creates the data dep), into a PSUM bank whose
            # first real use is late.  The first transpose can only reach
            # the PE ~33us in (the Activation hwdge queue starts ~32us
            # into every kernel), so the warm-up spans ~21-30us and the
            # HAM clock-gate is at 8/8 right when real matmuls start ----
            dummy = pool_w.tile([P, OC], f16, tag="warm", name="warm")

            def emit_warm(q32_gate):
                nc.gpsimd.memset(dummy[:], 0.0)
                nc.gpsimd.tensor_copy(out=dummy[:, :P], in_=q32_gate[:, :P])
                wps = pool_ps.tile([P, OC], f32, tag="ps1_3", name="warmps")
                for i in range(warm_mms):
                    nc.tensor.matmul(
                        wps[:], dummy[:, :P], dummy[:],
                        start=(i == 0), stop=(i == warm_mms - 1),
                    )

            # ---- quant: per chunk (halves; quarters for tile 0) — stats
            # from fp16, scale into fp32 scratch, flat per-engine round,
            # dequant fp32->fp16 back into the x buffer, XBAR transpose.
            # NO ACT-engine instructions anywhere in the kernel: the ACT
            # table load blocks the Activation queue until ~40us, so the
            # scalar queue must stay pure-DMA (transposes + output) ----
            def rnd(eng, q32, c0, c1):   # exact fp32 RNE via +/-MAGIC, flat
                eng.tensor_scalar(
                    out=q32[:, c0:c1], in0=q32[:, c0:c1],
                    scalar1=MAGIC, scalar2=MAGIC,
                    op0=mybir.AluOpType.add, op1=mybir.AluOpType.subtract,
                )

            def emit_quant(t, xq_=None, nch=2, after_chunk0=None,
                           do_transpose=True):
                if xq_ is None:
                    xq_ = pool_x.tile([P, K], f16, tag="x", name=f"x{t}")
                    nc.gpsimd.dma_start(out=xq_[:], in_=x[t * P : (t + 1) * P, :])
                xts_t = pool_xt.tile([P, KT, P], f16, tag="xt", name=f"xts{t}")
                amax = pool_s.tile([P, G], f32, tag="amax", name=f"amax{t}")
                delta = pool_s.tile([P, G], f32, tag="delta", name=f"delta{t}")
                recip = pool_s.tile([P, G], f32, tag="recip", name=f"recip{t}")
                xr = xq_.rearrange("p (g v) -> p g v", v=V)
                CH = K // nch            # cols per chunk
                GC = G // nch            # groups per chunk
                KTC = KT // nch          # k-tiles per chunk
                GVC = (GVH * 2) // nch   # vector-side groups per chunk

                for h in range(nch):
                    g0 = h * GC
                    gs = slice(g0, g0 + GC)
                    # stats straight off the fp16 tile
                    nc.vector.tensor_reduce(
                        out=amax[:, gs], in_=xr[:, gs, :], axis=mybir.AxisListType.X,
                        op=amax_op, apply_absolute_value=True,
                    )
                    nc.vector.tensor_scalar(
                        out=delta[:, gs], in0=amax[:, gs],
                        scalar1=QSCALE, scalar2=DELTA_MIN, op0=mult,
                        op1=amax_op,
                    )
                    nc.vector.reciprocal(out=recip[:, gs], in_=delta[:, gs])

                    # column-split scale/dequant on vector+gpsimd; the
                    # fused +/-MAGIC round runs as ONE flat op on vector
                    # covering the whole chunk (the 2-op tensor_scalar is
                    # pathologically slow on gpsimd, ~15 ns/col, and drags
                    # concurrent vector ops with it)
                    q32 = pool_q.tile([P, H], f32, tag="q", name=f"q{t}_{h}")
                    qr = q32.rearrange("p (g v) -> p g v", v=V)

                    def rmul(eng, l0, l1):   # q32 = x16 * (1/delta)
                        eng.tensor_tensor(
                            out=qr[:, l0:l1, :], in0=xr[:, g0 + l0 : g0 + l1, :],
                            in1=recip[:, g0 + l0 : g0 + l1, None].to_broadcast(
                                (P, l1 - l0, V)), op=mult,
                        )

                    def dmul(eng, l0, l1):   # x16 = round(q32) * delta
                        eng.tensor_tensor(
                            out=xr[:, g0 + l0 : g0 + l1, :], in0=qr[:, l0:l1, :],
                            in1=delta[:, g0 + l0 : g0 + l1, None].to_broadcast(
                                (P, l1 - l0, V)), op=mult,
                        )

                    rmul(nc.vector, 0, GVC)
                    rmul(nc.gpsimd, GVC, GC)
                    rnd(nc.vector, q32, 0, GC * V)
                    dmul(nc.vector, 0, GVC)
                    dmul(nc.gpsimd, GVC, GC)
                    if do_transpose:
                        nc.scalar.dma_start_transpose(
                            xts_t[:, h * KTC : (h + 1) * KTC, :],
                            xq_[:, h * CH : (h + 1) * CH],
                        )
                    if h == 0 and after_chunk0 is not None:
                        after_chunk0(q32)
                return xts_t, xq_

            # ---- PSUM evac: DVE copy (gpsimd has no PSUM port), then
            # output DMA on the scalar queue (sync queue stays pure-W) ----
            def evac(t, oc, ps):
                ot = pool_o.tile([P, OC], f32, tag="o", name=f"ot{t}_{oc}")
                nc.vector.tensor_copy(out=ot[:], in_=ps[:])
                nc.scalar.dma_start(
                    out=out[t * P : (t + 1) * P, oc * OC : (oc + 1) * OC], in_=ot[:]
                )

            def emit_mm(t, xts_t, ocs, mode):
                if mode == "oc":
                    for oc in ocs:
                        ps = pool_ps.tile([P, OC], f32, tag=f"ps{t % 2}_{oc}",
                                          name=f"ps{t}_{oc}")
                        for kt in range(KT):
                            nc.tensor.matmul(
                                ps[:], xts_t[:, kt, :], wcur[oc][:, kt, :],
                                start=(kt == 0), stop=(kt == KT - 1),
                            )
                        evac(t, oc, ps)
                else:
                    pss = {
                        oc: pool_ps.tile([P, OC], f32, tag=f"ps{t % 2}_{oc}",
                                         name=f"ps{t}_{oc}")
                        for oc in ocs
                    }
                    for kt in range(KT):
                        for oc in ocs:
                            nc.tensor.matmul(
                                pss[oc][:], xts_t[:, kt, :], wcur[oc][:, kt, :],
                                start=(kt == 0), stop=(kt == KT - 1),
                            )
                    for oc in ocs:
                        evac(t, oc, pss[oc])

            # The first `stag` token tiles run oc-pair-staggered so demand
            # for W chunks 2/3 starts only ~62/69us in; "oc" mode on the
            # first segment of each pair lets matmuls start on partially-
            # arrived chunks.  Last tile oc-major to shrink the evac tail.
            ALL = list(range(NOC))
            segs = []
            for pair in ([0, 1], [2, 3]):
                for t in range(stag):
                    segs.append((t, pair, "oc" if t == 0 else "k"))
            for t in range(stag, NT - 1):
                segs.append((t, ALL, "k"))
            segs.append((NT - 1, ALL, "oc"))

            tiles = {0: emit_quant(0, xq0, nch=4, after_chunk0=emit_warm)}
            qnext = 1

            def emit_q_upto(n):
                nonlocal qnext
                while qnext < min(n, NT):
                    # tiles >= stag quant early (x-pool paced) but their
                    # transposes are DEFERRED: emitted inline they would
                    # wait on xts buffers released only by the [3]-phase
                    # segments and head-block the scalar ring
                    tiles[qnext] = emit_quant(qnext)
                    qnext += 1

            emit_q_upto(2)
            for si, (t, ocs, mode) in enumerate(segs):
                if t >= qnext:
                    emit_q_upto(t + 1)
                emit_mm(t, tiles[t][0], ocs, mode)
                emit_q_upto(3 + si)

    if split:
        _split_multiwait(nc)
    return nc


_CACHED = {}

# test-harness knobs (kernel() defaults are what the grader uses)
TRACE = False
LAST_RESULT = None
BUILD_KW = {}


def _get_nc(shape_key):
    if shape_key not in _CACHED:
        T, K, O = shape_key
        _CACHED[shape_key] = build(T=T, K=K, O=O, **BUILD_KW)
    return _CACHED[shape_key]


def pack_w(W: np.ndarray, OC: int = 512, P: int = 128) -> np.ndarray:
    # [out,in] -> W^T [in,out] fp16, packed [NOC, P, KT*OC] so each per-core
    # o-chunk W load is one fully contiguous DMA
    K, O = W.shape[1], W.shape[0]
    KT, NOC = K // P, O // OC
    wt = np.ascontiguousarray(W.T).astype(np.float16)         # [K, O]
    z = wt.reshape(KT, P, NOC, OC).transpose(2, 1, 0, 3)      # [NOC, P, KT, OC]
    return np.ascontiguousarray(z.reshape(NOC, P, KT * OC))


def kernel(x: np.ndarray, W: np.ndarray, b: np.ndarray) -> np.ndarray:
    global LAST_RESULT
    n, k = x.shape               # 8192, 4096
    o = W.shape[0]               # 4096
    assert n % TP == 0 and o % OP == 0
    tpc = n // TP                # 2048 tokens per core
    osh = o // OP                # 2048 out-features per core
    nc = _get_nc((tpc, k, osh))

    wtp = pack_w(W)              # [8, 128, 16384]
    ncs = osh // 512             # oc chunks per shard (4)
    xs = np.ascontiguousarray(x.astype(np.float16)).reshape(TP, tpc, k)
    in_maps = []
    for i in range(N_CORES):
        tb, ob = divmod(i, OP)
        in_maps.append(
            {"x": xs[tb], "wt": np.ascontiguousarray(wtp[ob * ncs : (ob + 1) * ncs])}
        )
    res = run_bass_kernel_spmd(nc, in_maps, list(range(N_CORES)), trace=TRACE)
    LAST_RESULT = res
    full = np.empty((n, o), np.float32)
    for i in range(N_CORES):
        tb, ob = divmod(i, OP)
        full[tb * tpc : (tb + 1) * tpc, ob * osh : (ob + 1) * osh] = (
            res.results[i]["out"]
        )
    full += b.astype(np.float32)[None, :]
    return full


# revision 27
# speedup vs baseline: 1.2601x; 1.0085x over previous
"""Trainium2 Bass kernel for group-quant (fake int8, V=64) + Linear.

reference math (per row of x):
    absmax over feature-groups of 64 -> delta = max(2*absmax/254, 1e-5)
    xq = clip(round(x/delta), -127, 127) * delta      (fake quant)
    out = xq @ W.T + b

Sharding (v6): 2-D — tokens 4-way x out-features 2-way across 8 cores.
Each core handles 2048 tokens x 2048 out-features; its W^T shard
([4096, 2048] fp16, pre-packed on host) is 128 KB/partition and stays
FULLY resident in SBUF: single phase, no W re-load, no x~^T spill.

x is cast to fp16 on the host: |x| <= ~6 so fp16's 10-bit mantissa
keeps the quant decisions almost always identical to fp32 (measured
end-to-end rel err 2.5e-3 vs the 2e-2 budget).  This halves x HBM
traffic to 16.8 MB/core — the first ~100us are DMA-engine bound (the
W shard + x + XBAR transposes saturate the ~358 GB/s per-core fabric),
so x bytes trade 1:1 against how fast W can land.  The quant ARITHMETIC
stays fp32: strided/broadcast DVE ops on fp16 measured 2-10x SLOWER
than fp32 (no 16-bit fast path for those APs), so each half is scaled
fp16->fp32 into a scratch (upconvert fused into the multiply), rounded
flat on the ACT engine (2 Copy+bias passes; fp32 internal math makes
+/-1.5*2^23 an exact RNE), and dequanted fp32->fp16 back into the x
buffer (that mixed broadcast pattern measured fast), then transposed.

Device schedule per core:
  A burst of dummy matmuls, gated on the first x tile's DMA (so the
  HAM clock-gate warm-up isn't wasted waiting), brings the PE to full
  clock right before real matmuls start.  Matmuls run k-outer/oc-inner
  (4 matmuls share one stationary), accumulating into 4 PSUM banks
  ping-ponged across token tiles.  The first four token tiles run
  oc-pair-staggered segments so the W stream (sync queue, nothing else
  on it) stays ahead of demand: chunks 2/3 are not needed until
  ~62/69us.  PSUM is evacuated by ACT copies; output DMAs follow on
  the scalar queue (bias added on host).
"""

import numpy as np

import concourse.bass as bass
import concourse.mybir as mybir
import concourse.tile as tile
from concourse.bass_utils import run_bass_kernel_spmd

N_CORES = 8
TP = 4                     # token-parallel ways
OP = 2                     # out-feature-parallel ways
MAGIC = 1.5 * 2.0**23      # fp32 round-to-nearest-even constant
QSCALE = 1.0 / 127.0       # 2/(qmax-qmin) with qmax=127, qmin=-127
DELTA_MIN = 1e-5


def _split_multiwait(nc):
    """This walrus build allows at most ONE sync wait per instruction
    ("Too many sync wait commands", CoreV3GenImpl setupSyncWait) and none
    on Drain. Tile freely attaches several waits to one instruction, so
    post-process: move excess waits onto single-wait NoOps inserted just
    before the instruction on the same engine queue (semantics identical —
    the queue stalls at the nop instead of at the instruction)."""
    nid = 0
    for fn in nc.m.functions:
        for bb in fn.blocks:
            insts = list(bb.instructions)
            out = []
            changed = False
            for inst in insts:
                si = inst.sync_info
                waits = list(si.on_wait) if si is not None and si.on_wait else []
                limit = 0 if type(inst).__name__ == "InstDrain" else 1
                if len(waits) > limit:
                    changed = True
                    keep = waits[len(waits) - limit :] if limit else []
                    for w in waits[: len(waits) - limit]:
                        nid += 1
                        out.append(
                            mybir.InstNoOp(
                                name=f"WSPLIT-{nid}",
                                engine=inst.engine,
                                bass_nofuse=True,
                                ins=[],
                                outs=[],
                                sync_info=mybir.SyncInfo(on_wait=[w], on_update=[]),
                            )
                        )
                    si.on_wait = keep
                out.append(inst)
            if changed:
                try:
                    bb.instructions = out
                except Exception:
                    bb.instructions[:] = out


def build(T=2048, K=4096, O=2048, V=64, GVH=12, wq_split=4, split=True,
          warm_mms=40, stag=4):
    f32, f16 = mybir.dt.float32, mybir.dt.float16
    P = 128
    G = K // V                 # quant groups per row (64)
    GH = G // 2                # groups per half (32)
    H = K // 2                 # cols per half (2048)
    KT = K // P                # contraction tiles (32)
    NT = T // P                # token tiles per core (16)
    OC = 512                   # oc chunk (psum bank width fp32)
    NOC = O // OC              # 4
    KQW = KT // wq_split       # k-tiles per W DMA quarter (8)

    nc = bass.Bass()
    x = nc.dram_tensor("x", [T, K], f16, kind="ExternalInput")
    wt = nc.dram_tensor("wt", [NOC, P, KT * OC], f16, kind="ExternalInput")
    out = nc.dram_tensor("out", [T, O], f32, kind="ExternalOutput")

    mult = mybir.AluOpType.mult
    amax_op = mybir.AluOpType.max

    with tile.TileContext(nc) as tc:
        with (
            tc.tile_pool(name="x", bufs=2) as pool_x,
            tc.tile_pool(name="q32", bufs=3) as pool_q,
            tc.tile_pool(name="st", bufs=2) as pool_s,
            tc.tile_pool(name="xt", bufs=4) as pool_xt,
            tc.tile_pool(name="w", bufs=1) as pool_w,
            tc.tile_pool(name="o", bufs=2) as pool_o,
            tc.tile_pool(name="ps", bufs=1, space="PSUM") as pool_ps,
        ):
            # ---- W shard loads: sync queue carries ONLY these ----
            def post_w(oc):
                wtile = pool_w.tile([P, KT, OC], f16, tag=f"w{oc}", name=f"w{oc}")
                for q in range(wq_split):
                    nc.sync.dma_start(
                        out=wtile[:, q * KQW : (q + 1) * KQW, :].rearrange(
                            "p k o -> p (k o)"
                        ),
                        in_=wt[oc][:, q * KQW * OC : (q + 1) * KQW * OC],
                    )
                return wtile

            wcur = [post_w(oc) for oc in range(NOC)]

            # ---- quant tile 0 DMA first (quarters, so its chain starts
            # the moment the first 0.25 MB lands) ----
            xq0 = pool_x.tile([P, K], f16, tag="x", name="x0")
            Q4 = K // 4
            for qq in range(4):
                nc.gpsimd.dma_start(
                    out=xq0[:, qq * Q4 : (qq + 1) * Q4],
                    in_=x[0:P, qq * Q4 : (qq + 1) * Q4],
                )

            # ---- PE warm-up: dummy matmuls gated on tile 0's first quant
            # chunk (copy creates the data dep), into a PSUM bank whose
            # first real use is late.  The first transpose can only reach
            # the PE ~33us in (the Activation hwdge queue starts ~32us
            # into every kernel), so the warm-up spans ~21-30us and the
            # HAM clock-gate is at 8/8 right when real matmuls start ----
            dummy = pool_w.tile([P, OC], f16, tag="warm", name="warm")

            def emit_warm(q32_gate):
                nc.gpsimd.memset(dummy[:], 0.0)
                nc.gpsimd.tensor_copy(out=dummy[:, :P], in_=q32_gate[:, :P])
                wps = pool_ps.tile([P, OC], f32, tag="ps1_3", name="warmps")
                for i in range(warm_mms):
                    nc.tensor.matmul(
                        wps[:], dummy[:, :P], dummy[:],
                        start=(i == 0), stop=(i == warm_mms - 1),
                    )

            # ---- quant: per chunk (halves; quarters for tile 0) — stats
            # from fp16, scale into fp32 scratch, flat per-engine round,
            # dequant fp32->fp16 back into the x buffer, XBAR transpose.
            # NO ACT-engine instructions anywhere in the kernel: the ACT
            # table load blocks the Activation queue until ~40us, so the
            # scalar queue must stay pure-DMA (transposes + output) ----
            def rnd(eng, q32, c0, c1):   # exact fp32 RNE via +/-MAGIC, flat
                eng.tensor_scalar(
                    out=q32[:, c0:c1], in0=q32[:, c0:c1],
                    scalar1=MAGIC, scalar2=MAGIC,
                    op0=mybir.AluOpType.add, op1=mybir.AluOpType.subtract,
                )

            def emit_quant(t, xq_=None, nch=2, after_chunk0=None,
                           do_transpose=True):
                if xq_ is None:
                    xq_ = pool_x.tile([P, K], f16, tag="x", name=f"x{t}")
                    nc.gpsimd.dma_start(out=xq_[:], in_=x[t * P : (t + 1) * P, :])
                xts_t = pool_xt.tile([P, KT, P], f16, tag="xt", name=f"xts{t}")
                amax = pool_s.tile([P, G], f32, tag="amax", name=f"amax{t}")
                delta = pool_s.tile([P, G], f32, tag="delta", name=f"delta{t}")
                recip = pool_s.tile([P, G], f32, tag="recip", name=f"recip{t}")
                xr = xq_.rearrange("p (g v) -> p g v", v=V)
                CH = K // nch            # cols per chunk
                GC = G // nch            # groups per chunk
                KTC = KT // nch          # k-tiles per chunk
                GVC = (GVH * 2) // nch   # vector-side groups per chunk

                for h in range(nch):
                    g0 = h * GC
                    gs = slice(g0, g0 + GC)
                    # stats straight off the fp16 tile
                    nc.vector.tensor_reduce(
                        out=amax[:, gs], in_=xr[:, gs, :], axis=mybir.AxisListType.X,
                        op=amax_op, apply_absolute_value=True,
                    )
                    nc.vector.tensor_scalar(
                        out=delta[:, gs], in0=amax[:, gs],
                        scalar1=QSCALE, scalar2=DELTA_MIN, op0=mult,
                        op1=amax_op,
                    )
                    nc.vector.reciprocal(out=recip[:, gs], in_=delta[:, gs])

                    # column-split scale/dequant on vector+gpsimd; the
                    # fused +/-MAGIC round runs as ONE flat op on vector
                    # covering the whole chunk (the 2-op tensor_scalar is
                    # pathologically slow on gpsimd, ~15 ns/col, and drags
                    # concurrent vector ops with it)
                    q32 = pool_q.tile([P, H], f32, tag="q", name=f"q{t}_{h}")
                    qr = q32.rearrange("p (g v) -> p g v", v=V)

                    def rmul(eng, l0, l1):   # q32 = x16 * (1/delta)
                        eng.tensor_tensor(
                            out=qr[:, l0:l1, :], in0=xr[:, g0 + l0 : g0 + l1, :],
                            in1=recip[:, g0 + l0 : g0 + l1, None].to_broadcast(
                                (P, l1 - l0, V)), op=mult,
                        )

                    def dmul(eng, l0, l1):   # x16 = round(q32) * delta
                        eng.tensor_tensor(
                            out=xr[:, g0 + l0 : g0 + l1, :], in0=qr[:, l0:l1, :],
                            in1=delta[:, g0 + l0 : g0 + l1, None].to_broadcast(
                                (P, l1 - l0, V)), op=mult,
                        )

                    rmul(nc.vector, 0, GVC)
                    rmul(nc.gpsimd, GVC, GC)
                    rnd(nc.vector, q32, 0, GC * V)
                    dmul(nc.vector, 0, GVC)
                    dmul(nc.gpsimd, GVC, GC)
                    if do_transpose:
                        nc.scalar.dma_start_transpose(
                            xts_t[:, h * KTC : (h + 1) * KTC, :],
                            xq_[:, h * CH : (h + 1) * CH],
                        )
                    if h == 0 and after_chunk0 is not None:
                        after_chunk0(q32)
                return xts_t, xq_

            # ---- PSUM evac: DVE copy (gpsimd has no PSUM port), then
            # output DMA on the scalar queue (sync queue stays pure-W) ----
            def evac(t, oc, ps):
                ot = pool_o.tile([P, OC], f32, tag="o", name=f"ot{t}_{oc}")
                nc.vector.tensor_copy(out=ot[:], in_=ps[:])
                nc.scalar.dma_start(
                    out=out[t * P : (t + 1) * P, oc * OC : (oc + 1) * OC], in_=ot[:]
                )

            def emit_mm(t, xts_t, ocs, mode):
                if mode == "oc":
                    for oc in ocs:
                        ps = pool_ps.tile([P, OC], f32, tag=f"ps{t % 2}_{oc}",
                                          name=f"ps{t}_{oc}")
                        for kt in range(KT):
                            nc.tensor.matmul(
                                ps[:], xts_t[:, kt, :], wcur[oc][:, kt, :],
                                start=(kt == 0), stop=(kt == KT - 1),
                            )
                        evac(t, oc, ps)
                else:
                    pss = {
                        oc: pool_ps.tile([P, OC], f32, tag=f"ps{t % 2}_{oc}",
                                         name=f"ps{t}_{oc}")
                        for oc in ocs
                    }
                    for kt in range(KT):
                        for oc in ocs:
                            nc.tensor.matmul(
                                pss[oc][:], xts_t[:, kt, :], wcur[oc][:, kt, :],
                                start=(kt == 0), stop=(kt == KT - 1),
                            )
                    for oc in ocs:
                        evac(t, oc, pss[oc])

            # The first `stag` token tiles run oc-pair-staggered so demand
            # for W chunks 2/3 starts only ~62/69us in; "oc" mode on the
            # first segment of each pair lets matmuls start on partially-
            # arrived chunks.  Last tile oc-major to shrink the evac tail.
            ALL = list(range(NOC))
            segs = []
            for pair in ([0, 1], [2, 3]):
                for t in range(stag):
                    segs.append((t, pair, "oc" if t == 0 else "k"))
            for t in range(stag, NT - 1):
                segs.append((t, ALL, "k"))
            segs.append((NT - 1, ALL, "oc"))

            tiles = {0: emit_quant(0, xq0, nch=4, after_chunk0=emit_warm)}
            qnext = 1

            def emit_q_upto(n):
                nonlocal qnext
                while qnext < min(n, NT):
                    # tiles >= stag quant early (x-pool paced) but their
                    # transposes are DEFERRED: emitted inline they would
                    # wait on xts buffers released only by the [3]-phase
                    # segments and head-block the scalar ring
                    tiles[qnext] = emit_quant(qnext)
                    qnext += 1

            emit_q_upto(2)
            for si, (t, ocs, mode) in enumerate(segs):
                if t >= qnext:
                    emit_q_upto(t + 1)
                emit_mm(t, tiles[t][0], ocs, mode)
                emit_q_upto(3 + si)

    if split:
        _split_multiwait(nc)
    return nc


_CACHED = {}

# test-harness knobs (kernel() defaults are what the grader uses)
TRACE = False
LAST_RESULT = None
BUILD_KW = {}


def _get_nc(shape_key):
    if shape_key not in _CACHED:
        T, K, O = shape_key
        _CACHED[shape_key] = build(T=T, K=K, O=O, **BUILD_KW)
    return _CACHED[shape_key]


def pack_w(W: np.ndarray, OC: int = 512, P: int = 128) -> np.ndarray:
    # [out,in] -> W^T [in,out] fp16, packed [NOC, P, KT*OC] so each per-core
    # o-chunk W load is one fully contiguous DMA
    K, O = W.shape[1], W.shape[0]
    KT, NOC = K // P, O // OC
    wt = np.ascontiguousarray(W.T).astype(np.float16)         # [K, O]
    z = wt.reshape(KT, P, NOC, OC).transpose(2, 1, 0, 3)      # [NOC, P, KT, OC]
    return np.ascontiguousarray(z.reshape(NOC, P, KT * OC))


def kernel(x: np.ndarray, W: np.ndarray, b: np.ndarray) -> np.ndarray:
    global LAST_RESULT
    n, k = x.shape               # 8192, 4096
    o = W.shape[0]               # 4096
    assert n % TP == 0 and o % OP == 0
    tpc = n // TP                # 2048 tokens per core
    osh = o // OP                # 2048 out-features per core
    nc = _get_nc((tpc, k, osh))

    wtp = pack_w(W)              # [8, 128, 16384]
    ncs = osh // 512             # oc chunks per shard (4)
    xs = np.ascontiguousarray(x.astype(np.float16)).reshape(TP, tpc, k)
    in_maps = []
    for i in range(N_CORES):
        tb, ob = divmod(i, OP)
        in_maps.append(
            {"x": xs[tb], "wt": np.ascontiguousarray(wtp[ob * ncs : (ob + 1) * ncs])}
        )
    res = run_bass_kernel_spmd(nc, in_maps, list(range(N_CORES)), trace=TRACE)
    LAST_RESULT = res
    full = np.empty((n, o), np.float32)
    for i in range(N_CORES):
        tb, ob = divmod(i, OP)
        full[tb * tpc : (tb + 1) * tpc, ob * osh : (ob + 1) * osh] = (
            res.results[i]["out"]
        )
    full += b.astype(np.float32)[None, :]
    return full


# revision 28
# speedup vs baseline: 1.2646x; 1.0036x over previous
"""Trainium2 Bass kernel for group-quant (fake int8, V=64) + Linear.

reference math (per row of x):
    absmax over feature-groups of 64 -> delta = max(2*absmax/254, 1e-5)
    xq = clip(round(x/delta), -127, 127) * delta      (fake quant)
    out = xq @ W.T + b

Sharding (v6): 2-D — tokens 4-way x out-features 2-way across 8 cores.
Each core handles 2048 tokens x 2048 out-features; its W^T shard
([4096, 2048] fp16, pre-packed on host) is 128 KB/partition and stays
FULLY resident in SBUF: single phase, no W re-load, no x~^T spill.

x is cast to fp16 on the host: |x| <= ~6 so fp16's 10-bit mantissa
keeps the quant decisions almost always identical to fp32 (measured
end-to-end rel err 2.5e-3 vs the 2e-2 budget).  This halves x HBM
traffic to 16.8 MB/core — the first ~100us are DMA-engine bound (the
W shard + x + XBAR transposes saturate the ~358 GB/s per-core fabric),
so x bytes trade 1:1 against how fast W can land.  The quant ARITHMETIC
stays fp32: strided/broadcast DVE ops on fp16 measured 2-10x SLOWER
than fp32 (no 16-bit fast path for those APs), so each half is scaled
fp16->fp32 into a scratch (upconvert fused into the multiply), rounded
flat on the ACT engine (2 Copy+bias passes; fp32 internal math makes
+/-1.5*2^23 an exact RNE), and dequanted fp32->fp16 back into the x
buffer (that mixed broadcast pattern measured fast), then transposed.

Device schedule per core:
  A burst of dummy matmuls, gated on the first x tile's DMA (so the
  HAM clock-gate warm-up isn't wasted waiting), brings the PE to full
  clock right before real matmuls start.  Matmuls run k-outer/oc-inner
  (4 matmuls share one stationary), accumulating into 4 PSUM banks
  ping-ponged across token tiles.  The first four token tiles run
  oc-pair-staggered segments so the W stream (sync queue, nothing else
  on it) stays ahead of demand: chunks 2/3 are not needed until
  ~62/69us.  PSUM is evacuated by ACT copies; output DMAs follow on
  the scalar queue (bias added on host).
"""

import numpy as np

import concourse.bass as bass
import concourse.mybir as mybir
import concourse.tile as tile
from concourse.bass_utils import run_bass_kernel_spmd

N_CORES = 8
TP = 4                     # token-parallel ways
OP = 2                     # out-feature-parallel ways
MAGIC = 1.5 * 2.0**23      # fp32 round-to-nearest-even constant
QSCALE = 1.0 / 127.0       # 2/(qmax-qmin) with qmax=127, qmin=-127
DELTA_MIN = 1e-5


def _split_multiwait(nc):
    """This walrus build allows at most ONE sync wait per instruction
    ("Too many sync wait commands", CoreV3GenImpl setupSyncWait) and none
    on Drain. Tile freely attaches several waits to one instruction, so
    post-process: move excess waits onto single-wait NoOps inserted just
    before the instruction on the same engine queue (semantics identical —
    the queue stalls at the nop instead of at the instruction)."""
    nid = 0
    for fn in nc.m.functions:
        for bb in fn.blocks:
            insts = list(bb.instructions)
            out = []
            changed = False
            for inst in insts:
                si = inst.sync_info
                waits = list(si.on_wait) if si is not None and si.on_wait else []
                limit = 0 if type(inst).__name__ == "InstDrain" else 1
                if len(waits) > limit:
                    changed = True
                    keep = waits[len(waits) - limit :] if limit else []
                    for w in waits[: len(waits) - limit]:
                        nid += 1
                        out.append(
                            mybir.InstNoOp(
                                name=f"WSPLIT-{nid}",
                                engine=inst.engine,
                                bass_nofuse=True,
                                ins=[],
                                outs=[],
                                sync_info=mybir.SyncInfo(on_wait=[w], on_update=[]),
                            )
                        )
                    si.on_wait = keep
                out.append(inst)
            if changed:
                try:
                    bb.instructions = out
                except Exception:
                    bb.instructions[:] = out


def build(T=2048, K=4096, O=2048, V=64, GVH=12, wq_split=4, split=True,
          warm_mms=40, stag=4):
    f32, f16 = mybir.dt.float32, mybir.dt.float16
    P = 128
    G = K // V                 # quant groups per row (64)
    GH = G // 2                # groups per half (32)
    H = K // 2                 # cols per half (2048)
    KT = K // P                # contraction tiles (32)
    NT = T // P                # token tiles per core (16)
    OC = 512                   # oc chunk (psum bank width fp32)
    NOC = O // OC              # 4
    KQW = KT // wq_split       # k-tiles per W DMA quarter (8)

    nc = bass.Bass()
    x = nc.dram_tensor("x", [T, K], f16, kind="ExternalInput")
    wt = nc.dram_tensor("wt", [NOC, P, KT * OC], f16, kind="ExternalInput")
    out = nc.dram_tensor("out", [T, O], f32, kind="ExternalOutput")

    mult = mybir.AluOpType.mult
    amax_op = mybir.AluOpType.max

    with tile.TileContext(nc) as tc:
        with (
            tc.tile_pool(name="x", bufs=2) as pool_x,
            tc.tile_pool(name="q32", bufs=3) as pool_q,
            tc.tile_pool(name="st", bufs=2) as pool_s,
            tc.tile_pool(name="xt", bufs=4) as pool_xt,
            tc.tile_pool(name="w", bufs=1) as pool_w,
            tc.tile_pool(name="o", bufs=2) as pool_o,
            tc.tile_pool(name="ps", bufs=1, space="PSUM") as pool_ps,
        ):
            # ---- W shard loads: sync queue carries ONLY these ----
            def post_w(oc):
                wtile = pool_w.tile([P, KT, OC], f16, tag=f"w{oc}", name=f"w{oc}")
                for q in range(wq_split):
                    nc.sync.dma_start(
                        out=wtile[:, q * KQW : (q + 1) * KQW, :].rearrange(
                            "p k o -> p (k o)"
                        ),
                        in_=wt[oc][:, q * KQW * OC : (q + 1) * KQW * OC],
                    )
                return wtile

            wcur = [post_w(oc) for oc in range(NOC)]

            # ---- quant tile 0 DMA first (quarters, so its chain starts
            # the moment the first 0.25 MB lands) ----
            xq0 = pool_x.tile([P, K], f16, tag="x", name="x0")
            Q4 = K // 4
            for qq in range(4):
                nc.gpsimd.dma_start(
                    out=xq0[:, qq * Q4 : (qq + 1) * Q4],
                    in_=x[0:P, qq * Q4 : (qq + 1) * Q4],
                )

            # ---- PE warm-up: dummy matmuls gated on tile 0's first quant
            # chunk (copy creates the data dep), into a PSUM bank whose
            # first real use is late.  The first transpose can only reach
            # the PE ~33us in (the Activation hwdge queue starts ~32us
            # into every kernel), so the warm-up spans ~21-30us and the
            # HAM clock-gate is at 8/8 right when real matmuls start ----
            dummy = pool_w.tile([P, OC], f16, tag="warm", name="warm")

            def emit_warm(q32_gate):
                # gate on the (conservatively-tracked) x tile rather than
                # the chunk-0 scratch: the later trigger (~29us) makes the
                # warm-up span bridge exactly to the first transpose (~33us,
                # the Activation-queue start latency), so the PE never sits
                # idle past a HAM window before real matmuls begin
                nc.gpsimd.memset(dummy[:], 0.0)
                nc.gpsimd.tensor_copy(out=dummy[:, :P], in_=xq0[:, :P])
                wps = pool_ps.tile([P, OC], f32, tag="ps1_3", name="warmps")
                for i in range(warm_mms):
                    nc.tensor.matmul(
                        wps[:], dummy[:, :P], dummy[:],
                        start=(i == 0), stop=(i == warm_mms - 1),
                    )

            # ---- quant: per chunk (halves; quarters for tile 0) — stats
            # from fp16, scale into fp32 scratch, flat per-engine round,
            # dequant fp32->fp16 back into the x buffer, XBAR transpose.
            # NO ACT-engine instructions anywhere in the kernel: the ACT
            # table load blocks the Activation queue until ~40us, so the
            # scalar queue must stay pure-DMA (transposes + output) ----
            def rnd(eng, q32, c0, c1):   # exact fp32 RNE via +/-MAGIC, flat
                eng.tensor_scalar(
                    out=q32[:, c0:c1], in0=q32[:, c0:c1],
                    scalar1=MAGIC, scalar2=MAGIC,
                    op0=mybir.AluOpType.add, op1=mybir.AluOpType.subtract,
                )

            def emit_quant(t, xq_=None, nch=2, after_chunk0=None,
                           do_transpose=True):
                if xq_ is None:
                    xq_ = pool_x.tile([P, K], f16, tag="x", name=f"x{t}")
                    nc.gpsimd.dma_start(out=xq_[:], in_=x[t * P : (t + 1) * P, :])
                xts_t = pool_xt.tile([P, KT, P], f16, tag="xt", name=f"xts{t}")
                amax = pool_s.tile([P, G], f32, tag="amax", name=f"amax{t}")
                delta = pool_s.tile([P, G], f32, tag="delta", name=f"delta{t}")
                recip = pool_s.tile([P, G], f32, tag="recip", name=f"recip{t}")
                xr = xq_.rearrange("p (g v) -> p g v", v=V)
                CH = K // nch            # cols per chunk
                GC = G // nch            # groups per chunk
                KTC = KT // nch          # k-tiles per chunk
                GVC = (GVH * 2) // nch   # vector-side groups per chunk

                for h in range(nch):
                    g0 = h * GC
                    gs = slice(g0, g0 + GC)
                    # stats straight off the fp16 tile
                    nc.vector.tensor_reduce(
                        out=amax[:, gs], in_=xr[:, gs, :], axis=mybir.AxisListType.X,
                        op=amax_op, apply_absolute_value=True,
                    )
                    nc.vector.tensor_scalar(
                        out=delta[:, gs], in0=amax[:, gs],
                        scalar1=QSCALE, scalar2=DELTA_MIN, op0=mult,
                        op1=amax_op,
                    )
                    nc.vector.reciprocal(out=recip[:, gs], in_=delta[:, gs])

                    # column-split scale/dequant on vector+gpsimd; the
                    # fused +/-MAGIC round runs as ONE flat op on vector
                    # covering the whole chunk (the 2-op tensor_scalar is
                    # pathologically slow on gpsimd, ~15 ns/col, and drags
                    # concurrent vector ops with it)
                    q32 = pool_q.tile([P, H], f32, tag="q", name=f"q{t}_{h}")
                    qr = q32.rearrange("p (g v) -> p g v", v=V)

                    def rmul(eng, l0, l1):   # q32 = x16 * (1/delta)
                        eng.tensor_tensor(
                            out=qr[:, l0:l1, :], in0=xr[:, g0 + l0 : g0 + l1, :],
                            in1=recip[:, g0 + l0 : g0 + l1, None].to_broadcast(
                                (P, l1 - l0, V)), op=mult,
                        )

                    def dmul(eng, l0, l1):   # x16 = round(q32) * delta
                        eng.tensor_tensor(
                            out=xr[:, g0 + l0 : g0 + l1, :], in0=qr[:, l0:l1, :],
                            in1=delta[:, g0 + l0 : g0 + l1, None].to_broadcast(
                                (P, l1 - l0, V)), op=mult,
                        )

                    rmul(nc.vector, 0, GVC)
                    rmul(nc.gpsimd, GVC, GC)
                    rnd(nc.vector, q32, 0, GC * V)
                    dmul(nc.vector, 0, GVC)
                    dmul(nc.gpsimd, GVC, GC)
                    if do_transpose:
                        nc.scalar.dma_start_transpose(
                            xts_t[:, h * KTC : (h + 1) * KTC, :],
                            xq_[:, h * CH : (h + 1) * CH],
                        )
                    if h == 0 and after_chunk0 is not None:
                        after_chunk0(q32)
                return xts_t, xq_

            # ---- PSUM evac: DVE copy (gpsimd has no PSUM port), then
            # output DMA on the scalar queue (sync queue stays pure-W) ----
            def evac(t, oc, ps):
                ot = pool_o.tile([P, OC], f32, tag="o", name=f"ot{t}_{oc}")
                nc.vector.tensor_copy(out=ot[:], in_=ps[:])
                nc.scalar.dma_start(
                    out=out[t * P : (t + 1) * P, oc * OC : (oc + 1) * OC], in_=ot[:]
                )

            def emit_mm(t, xts_t, ocs, mode):
                if mode == "oc":
                    for oc in ocs:
                        ps = pool_ps.tile([P, OC], f32, tag=f"ps{t % 2}_{oc}",
                                          name=f"ps{t}_{oc}")
                        for kt in range(KT):
                            nc.tensor.matmul(
                                ps[:], xts_t[:, kt, :], wcur[oc][:, kt, :],
                                start=(kt == 0), stop=(kt == KT - 1),
                            )
                        evac(t, oc, ps)
                else:
                    pss = {
                        oc: pool_ps.tile([P, OC], f32, tag=f"ps{t % 2}_{oc}",
                                         name=f"ps{t}_{oc}")
                        for oc in ocs
                    }
                    for kt in range(KT):
                        for oc in ocs:
                            nc.tensor.matmul(
                                pss[oc][:], xts_t[:, kt, :], wcur[oc][:, kt, :],
                                start=(kt == 0), stop=(kt == KT - 1),
                            )
                    for oc in ocs:
                        evac(t, oc, pss[oc])

            # The first `stag` token tiles run oc-pair-staggered so demand
            # for W chunks 2/3 starts only ~62/69us in; "oc" mode on the
            # first segment of each pair lets matmuls start on partially-
            # arrived chunks.  Last tile oc-major to shrink the evac tail.
            ALL = list(range(NOC))
            segs = []
            for pair in ([0, 1], [2, 3]):
                for t in range(stag):
                    segs.append((t, pair, "oc" if t == 0 else "k"))
            for t in range(stag, NT - 1):
                segs.append((t, ALL, "k"))
            segs.append((NT - 1, ALL, "oc"))

            tiles = {0: emit_quant(0, xq0, nch=4, after_chunk0=emit_warm)}
            qnext = 1

            def emit_q_upto(n):
                nonlocal qnext
                while qnext < min(n, NT):
                    # tiles >= stag quant early (x-pool paced) but their
                    # transposes are DEFERRED: emitted inline they would
                    # wait on xts buffers released only by the [3]-phase
                    # segments and head-block the scalar ring
                    tiles[qnext] = emit_quant(qnext)
                    qnext += 1

            emit_q_upto(2)
            for si, (t, ocs, mode) in enumerate(segs):
                if t >= qnext:
                    emit_q_upto(t + 1)
                emit_mm(t, tiles[t][0], ocs, mode)
                emit_q_upto(3 + si)

    if split:
        _split_multiwait(nc)
    return nc


_CACHED = {}

# test-harness knobs (kernel() defaults are what the grader uses)
TRACE = False
LAST_RESULT = None
BUILD_KW = {}


def _get_nc(shape_key):
    if shape_key not in _CACHED:
        T, K, O = shape_key
        _CACHED[shape_key] = build(T=T, K=K, O=O, **BUILD_KW)
    return _CACHED[shape_key]


def pack_w(W: np.ndarray, OC: int = 512, P: int = 128) -> np.ndarray:
    # [out,in] -> W^T [in,out] fp16, packed [NOC, P, KT*OC] so each per-core
    # o-chunk W load is one fully contiguous DMA
    K, O = W.shape[1], W.shape[0]
    KT, NOC = K // P, O // OC
    wt = np.ascontiguousarray(W.T).astype(np.float16)         # [K, O]
    z = wt.reshape(KT, P, NOC, OC).transpose(2, 1, 0, 3)      # [NOC, P, KT, OC]
    return np.ascontiguousarray(z.reshape(NOC, P, KT * OC))


def kernel(x: np.ndarray, W: np.ndarray, b: np.ndarray) -> np.ndarray:
    global LAST_RESULT
    n, k = x.shape               # 8192, 4096
    o = W.shape[0]               # 4096
    assert n % TP == 0 and o % OP == 0
    tpc = n // TP                # 2048 tokens per core
    osh = o // OP                # 2048 out-features per core
    nc = _get_nc((tpc, k, osh))

    wtp = pack_w(W)              # [8, 128, 16384]
    ncs = osh // 512             # oc chunks per shard (4)
    xs = np.ascontiguousarray(x.astype(np.float16)).reshape(TP, tpc, k)
    in_maps = []
    for i in range(N_CORES):
        tb, ob = divmod(i, OP)
        in_maps.append(
            {"x": xs[tb], "wt": np.ascontiguousarray(wtp[ob * ncs : (ob + 1) * ncs])}
        )
    res = run_bass_kernel_spmd(nc, in_maps, list(range(N_CORES)), trace=TRACE)
    LAST_RESULT = res
    full = np.empty((n, o), np.float32)
    for i in range(N_CORES):
        tb, ob = divmod(i, OP)
        full[tb * tpc : (tb + 1) * tpc, ob * osh : (ob + 1) * osh] = (
            res.results[i]["out"]
        )
    full += b.astype(np.float32)[None, :]
    return full
